# revision 27
# baseline (speedup 1.0000x reference)
"""Bahdanau additive attention on 8 trn2 NeuronCores.

Reference computation (per batch b):
    Uh = enc[b] @ U_a                      # [Te, De]
    Ws = dec[b] @ W_a                      # [Td, De]
    e[t, j] = sum_d tanh(Uh[j, d] + Ws[t, d]) * V_a[d]    # [Td, Te]
    attn = softmax(e, axis=-1)
    ctx = attn @ enc[b]                    # [Td, De]
    returns (ctx, attn)

Sharding: data-parallel over (batch b, half of Td) -> 8 shards, each core
computes a [128, Te] attention block + [128, De] context block.

Per-core kernel layout ("layout B", d on partitions):
  - WsT[d, t] / UhT[d, j] computed on-chip (PE transposes + fp32 matmuls),
    then cast to fp16 for the argument build.
  - arg tiles [128d, T*JB] (t-major, j-minor) built by DVE tensor_add in
    fp16 2x mode: WsT pre-expanded (GPSIMD) so both operands have
    innermost step 1; UhT broadcast via a step-0 outer dim.
  - tanh on ACT in large instructions, written as float32r.
  - V-dot on PE: stationary = f32r V chunk [128d, 1] (M=1, trivial
    LDWEIGHTS), moving = contiguous f32r tanh slice [128d, 512] at
    1 cyc/row, d-chunks accumulated in PSUM rows [1, 512]. Rows are
    staged PSUM->SBUF by DVE copies software-pipelined 4 rows behind the
    matmuls (strict-FIFO engines never head-of-line block), then tiny
    SBUF->SBUF DMAs departition each row into E_sb[t, j].
  - softmax along free axis (ACT exp with accum_out + DVE reciprocal),
    context = attn @ enc via PE (attn transposed on PE).
  Arithmetic: everything feeding the tanh argument is fp32-exact except
  the final fp16 rounding of Ws/Uh/arg; the V-dot runs in float32r
  (~11-bit mantissa). End-to-end absmax error ~3e-4 relative to scale.
"""

import sys

if "/opt/trn_rl_repo" not in sys.path:
    sys.path.insert(0, "/opt/trn_rl_repo")

from contextlib import ExitStack

import numpy as np

import concourse.bass as bass
import concourse.mybir as mybir
import concourse.tile as tile
from concourse import bacc
from concourse.bass_utils import run_bass_kernel_spmd
from concourse.masks import make_identity

B, TE, TD, DE, DD = 4, 256, 256, 512, 512
NCORES = 8
T = 128          # decoder timesteps per core (TD=256 split in 2)
C = 4            # d-chunks of 128 (DE = 512)
JB = 16          # encoder positions per arg tile
NJB = TE // JB   # arg tiles per d-chunk

F32 = mybir.dt.float32
F16 = mybir.dt.float16
AF = mybir.ActivationFunctionType

_CACHE = {}


def _build_program(loop_iters=None):
    nc = bacc.Bacc("TRN2", target_bir_lowering=False, debug=False,
                   num_devices=NCORES)

    enc_d = nc.dram_tensor("enc", [TE, DE], F32, kind="ExternalInput").ap()
    dec_d = nc.dram_tensor("dec", [T, DD], F32, kind="ExternalInput").ap()
    ua_d = nc.dram_tensor("Ua", [DD, DE], F32, kind="ExternalInput").ap()
    wa_d = nc.dram_tensor("Wa", [DD, DE], F32, kind="ExternalInput").ap()
    va_d = nc.dram_tensor("Va", [DE], F32, kind="ExternalInput").ap()
    ctx_d = nc.dram_tensor("ctx_out", [T, DE], F32, kind="ExternalOutput").ap()
    attn_d = nc.dram_tensor("attn_out", [T, TE], F32, kind="ExternalOutput").ap()

    with tile.TileContext(nc) as tc, ExitStack() as ctx:
        if loop_iters is not None:
            ET = mybir.EngineType
            ctx.enter_context(tc.For_i(0, loop_iters, 1, hint_engines=(
                ET.PE, ET.Activation, ET.DVE, ET.Pool, ET.SP)))
        persist = ctx.enter_context(tc.tile_pool(name="persist", bufs=1))
        argp = ctx.enter_context(tc.tile_pool(name="argp", bufs=3))
        tanhp = ctx.enter_context(tc.tile_pool(name="tanhp", bufs=8))
        psum = ctx.enter_context(tc.tile_pool(name="psum", bufs=2, space="PSUM"))
        epsum = ctx.enter_context(tc.tile_pool(name="epsum", bufs=6, space="PSUM"))
        stagep = ctx.enter_context(tc.tile_pool(name="stagep", bufs=6))

        # ---- load inputs ----
        enc_sb = [persist.tile([128, DE], F32, tag=f"enc{j}", name=f"enc{j}") for j in range(2)]
        for j in range(2):
            nc.sync.dma_start(enc_sb[j][:], enc_d[j * 128:(j + 1) * 128, :])
        dec_sb = persist.tile([128, DD], F32, tag="dec", name="dec")
        nc.sync.dma_start(dec_sb[:], dec_d[:])
        ua_sb = [persist.tile([128, DE], F32, tag=f"ua{c}", name=f"ua{c}") for c in range(C)]
        wa_sb = [persist.tile([128, DE], F32, tag=f"wa{c}", name=f"wa{c}") for c in range(C)]
        for c in range(C):
            nc.sync.dma_start(ua_sb[c][:], ua_d[c * 128:(c + 1) * 128, :])
            nc.sync.dma_start(wa_sb[c][:], wa_d[c * 128:(c + 1) * 128, :])
        v_sb = persist.tile([128, C], F32, tag="v", name="v")
        nc.sync.dma_start(v_sb[:], va_d.rearrange("(c p) -> p c", p=128))

        ident = persist.tile([128, 128], F32, tag="ident", name="ident")
        make_identity(nc, ident[:])

        # ---- transposes: encT[c][cp, j] = enc[j, c*128+cp], decT likewise ----
        encT = [persist.tile([128, TE], F32, tag=f"encT{c}", name=f"encT{c}") for c in range(C)]
        decT = [persist.tile([128, T], F32, tag=f"decT{c}", name=f"decT{c}") for c in range(C)]
        for c in range(C):
            for j in range(2):
                tp = psum.tile([128, 128], F32, tag="ps", name="ps")
                nc.tensor.transpose(tp[:], enc_sb[j][:, c * 128:(c + 1) * 128],
                                    ident[:])
                nc.vector.tensor_copy(encT[c][:, j * 128:(j + 1) * 128], tp[:])
            tp = psum.tile([128, 128], F32, tag="ps", name="ps")
            nc.tensor.transpose(tp[:], dec_sb[:, c * 128:(c + 1) * 128], ident[:])
            nc.vector.tensor_copy(decT[c][:], tp[:])

        # ---- UhT[dc][dp, j] = Uh[j, dc*128+dp]; WsT[dc][dp, t] ----
        uhT = [persist.tile([128, TE], F16, tag=f"uhT{c}", name=f"uhT{c}") for c in range(C)]
        wsT = [persist.tile([128, T], F16, tag=f"wsT{c}", name=f"wsT{c}") for c in range(C)]
        for dc in range(C):
            pu = psum.tile([128, TE], F32, tag="ps", name="ps")
            for cc in range(C):
                nc.tensor.matmul(pu[:], ua_sb[cc][:, dc * 128:(dc + 1) * 128],
                                 encT[cc][:], start=(cc == 0), stop=(cc == C - 1))
            nc.vector.tensor_copy(uhT[dc][:], pu[:])
            pw = psum.tile([128, T], F32, tag="ps", name="ps")
            for cc in range(C):
                nc.tensor.matmul(pw[:], wa_sb[cc][:, dc * 128:(dc + 1) * 128],
                                 decT[cc][:], start=(cc == 0), stop=(cc == C - 1))
            nc.vector.tensor_copy(wsT[dc][:], pw[:])

        # ---- main loop: energies E[t, j] ----
        # fp16 args: WsT expanded (t-major, JB repeats) once per chunk on
        # GPSIMD so both DVE tensor_add operands have innermost step 1 ->
        # 2x_1P mode. ACT reads fp16 args, writes tanh as float32r; PE
        # V-dot: stationary = f32r V chunk [128d, 1], moving = f32r tanh
        # [128d, 512] (1 cyc/row), accumulating the 4 d-chunks in PSUM
        # rows [1, 512] (t-major), staged to E_sb[t, j] via DVE/ACT copy
        # + tiny SBUF->SBUF departition DMAs.
        v_r = persist.tile([128, C], mybir.dt.float32r, tag="v_r", name="v_r")
        nc.vector.tensor_copy(v_r[:], v_sb[:])
        wse = [persist.tile([128, T * JB], F16, tag=f"wse{c}", name=f"wse{c}")
               for c in range(C)]
        for c in range(C):
            nc.gpsimd.tensor_copy(
                wse[c][:].rearrange("p (t r) -> p t r", t=T),
                wsT[c][:].unsqueeze(2).to_broadcast([128, T, JB]))
        e_sb = persist.tile([128, TE], F32, tag="e_sb", name="e_sb")
        TB = 512 // JB            # t's per PE matmul (N = TB*JB = 512)
        pending = []
        nrow = 0

        def drain_row():
            nonlocal nrow
            e_row, jb0, tb0 = pending.pop(0)
            stage = stagep.tile([1, TB * JB], F32, tag="stage", name="stage")
            # alternate staging engine; copies are issued ~4 rows after
            # their matmuls so they never head-of-line block tanh/adds
            nc.vector.tensor_copy(stage[:], e_row[0:1, :])
            nrow += 1
            nc.sync.dma_start(
                e_sb[tb0 * TB:(tb0 + 1) * TB, jb0 * JB:(jb0 + 1) * JB],
                stage[:].rearrange("o (t j) -> o t j", t=TB))

        for jb in range(NJB):
            tanhs = []
            for c in range(C):
                argt = argp.tile([128, T * JB], F16, tag="arg", name="arg")
                a3 = argt[:].rearrange("p (t j) -> p t j", t=T)
                w3 = wse[c][:].rearrange("p (t r) -> p t r", t=T)
                u3 = (uhT[c][:, jb * JB:(jb + 1) * JB]
                      .unsqueeze(1).to_broadcast([128, T, JB]))
                nc.vector.tensor_add(a3, w3, u3)
                tanht = tanhp.tile([128, T * JB], mybir.dt.float32r,
                                   tag="tanh", name="tanh")
                nc.scalar.activation(tanht[:], argt[:], AF.Tanh)
                tanhs.append(tanht)
            for tb in range(T // TB):
                while len(pending) >= 4:
                    drain_row()
                e_row = epsum.tile([128, TB * JB], F32, tag="e_row",
                                   name="e_row")
                for c in range(C):
                    # contiguous [128, 512] slice: t in [tb*TB, (tb+1)*TB),
                    # all JB j's of this tile, j-minor
                    rhs = tanhs[c][:, tb * TB * JB:(tb + 1) * TB * JB]
                    nc.tensor.matmul(e_row[0:1, :], v_r[:, c:c + 1], rhs,
                                     start=(c == 0), stop=(c == C - 1))
                pending.append((e_row, jb, tb))
        while pending:
            drain_row()

        nmx = persist.tile([128, 1], F32, tag="nmx", name="nmx")
        nc.vector.tensor_reduce(nmx[:], e_sb[:], axis=mybir.AxisListType.X,
                                op=mybir.AluOpType.max, negate=True)
        p_sb = persist.tile([128, TE], F32, tag="p_sb", name="p_sb")
        ssum = persist.tile([128, 1], F32, tag="ssum", name="ssum")
        nc.scalar.activation(p_sb[:], e_sb[:], AF.Exp, bias=nmx[:],
                             accum_out=ssum[:])
        rs = persist.tile([128, 1], F32, tag="rs", name="rs")
        nc.vector.reciprocal(rs[:], ssum[:])
        attn_sb = persist.tile([128, TE], F32, tag="attn_sb", name="attn_sb")
        nc.vector.tensor_scalar_mul(attn_sb[:], p_sb[:], rs[:])
        nc.sync.dma_start(attn_d[:], attn_sb[:])

        # ---- context = attn @ enc ----
        attnT = [persist.tile([128, 128], F32, tag=f"attnT{j}", name=f"attnT{j}") for j in range(2)]
        for j in range(2):
            tp = psum.tile([128, 128], F32, tag="ps", name="ps")
            nc.tensor.transpose(tp[:], attn_sb[:, j * 128:(j + 1) * 128], ident[:])
            nc.vector.tensor_copy(attnT[j][:], tp[:])
        cps = psum.tile([128, DE], F32, tag="ps", name="ps")
        for j in range(2):
            nc.tensor.matmul(cps[:], attnT[j][:], enc_sb[j][:],
                             start=(j == 0), stop=(j == 1))
        ctx_sb = persist.tile([128, DE], F32, tag="ctx_sb", name="ctx_sb")
        nc.vector.tensor_copy(ctx_sb[:], cps[:])
        nc.sync.dma_start(ctx_d[:], ctx_sb[:])

    nc.compile()
    return nc


def _get_program(loop_iters=None):
    key = ("nc", loop_iters)
    if key not in _CACHE:
        _CACHE[key] = _build_program(loop_iters)
    return _CACHE[key]


def _make_in_maps(encoder_outputs, decoder_outputs, U_a, W_a, V_a):
    enc = np.ascontiguousarray(np.asarray(encoder_outputs, dtype=np.float32))
    dec = np.ascontiguousarray(np.asarray(decoder_outputs, dtype=np.float32))
    ua = np.ascontiguousarray(np.asarray(U_a, dtype=np.float32))
    wa = np.ascontiguousarray(np.asarray(W_a, dtype=np.float32))
    va = np.ascontiguousarray(np.asarray(V_a, dtype=np.float32))
    in_maps = []
    for core in range(NCORES):
        b, th = divmod(core, 2)
        in_maps.append({
            "enc": enc[b],
            "dec": np.ascontiguousarray(dec[b, th * T:(th + 1) * T]),
            "Ua": ua,
            "Wa": wa,
            "Va": va,
        })
    return in_maps


def run(encoder_outputs, decoder_outputs, U_a, W_a, V_a, **run_kwargs):
    """Compile (cached), run on 8 cores, gather. Returns (results, context, attn)."""
    nc = _get_program()
    in_maps = _make_in_maps(encoder_outputs, decoder_outputs, U_a, W_a, V_a)
    res = run_bass_kernel_spmd(nc, in_maps, core_ids=list(range(NCORES)),
                               **run_kwargs)
    context = np.empty((B, TD, DE), np.float32)
    attn = np.empty((B, TD, TE), np.float32)
    for core in range(NCORES):
        b, th = divmod(core, 2)
        context[b, th * T:(th + 1) * T] = res.results[core]["ctx_out"]
        attn[b, th * T:(th + 1) * T] = res.results[core]["attn_out"]
    return res, context, attn


def kernel(encoder_outputs, decoder_outputs, U_a, W_a, V_a):
    _, context, attn = run(encoder_outputs, decoder_outputs, U_a, W_a, V_a)
    return context, attn


# revision 30
# speedup vs baseline: 1.0030x; 1.0030x over previous
"""Bahdanau additive attention on 8 trn2 NeuronCores.

Reference computation (per batch b):
    Uh = enc[b] @ U_a                      # [Te, De]
    Ws = dec[b] @ W_a                      # [Td, De]
    e[t, j] = sum_d tanh(Uh[j, d] + Ws[t, d]) * V_a[d]    # [Td, Te]
    attn = softmax(e, axis=-1)
    ctx = attn @ enc[b]                    # [Td, De]
    returns (ctx, attn)

Sharding: data-parallel over (batch b, half of Td) -> 8 shards, each core
computes a [128, Te] attention block + [128, De] context block.

Per-core kernel layout ("layout B", d on partitions):
  - WsT[d, t] / UhT[d, j] computed on-chip (PE transposes + fp32 matmuls),
    then cast to fp16 for the argument build.
  - arg tiles [128d, T*JB] (t-major, j-minor) built by DVE tensor_add in
    fp16 2x mode: WsT pre-expanded (GPSIMD) so both operands have
    innermost step 1; UhT broadcast via a step-0 outer dim.
  - tanh on ACT in large instructions, written as float32r.
  - V-dot on PE: stationary = f32r V chunk [128d, 1] (M=1, trivial
    LDWEIGHTS), moving = contiguous f32r tanh slice [128d, 512] at
    1 cyc/row, d-chunks accumulated in PSUM rows [1, 512]. Rows are
    staged PSUM->SBUF by DVE copies software-pipelined 4 rows behind the
    matmuls (strict-FIFO engines never head-of-line block), then tiny
    SBUF->SBUF DMAs departition each row into E_sb[t, j].
  - softmax along free axis (ACT exp with accum_out + DVE reciprocal),
    context = attn @ enc via PE (attn transposed on PE).
  Arithmetic: everything feeding the tanh argument is fp32-exact except
  the final fp16 rounding of Ws/Uh/arg; the V-dot runs in float32r
  (~11-bit mantissa). End-to-end absmax error ~3e-4 relative to scale.
"""

import sys

if "/opt/trn_rl_repo" not in sys.path:
    sys.path.insert(0, "/opt/trn_rl_repo")

from contextlib import ExitStack

import numpy as np

import concourse.bass as bass
import concourse.mybir as mybir
import concourse.tile as tile
from concourse import bacc
from concourse.bass_utils import run_bass_kernel_spmd
from concourse.masks import make_identity

B, TE, TD, DE, DD = 4, 256, 256, 512, 512
NCORES = 8
T = 128          # decoder timesteps per core (TD=256 split in 2)
C = 4            # d-chunks of 128 (DE = 512)
JB = 16          # encoder positions per arg tile
NJB = TE // JB   # arg tiles per d-chunk

F32 = mybir.dt.float32
F16 = mybir.dt.float16
AF = mybir.ActivationFunctionType

_CACHE = {}


def _build_program(loop_iters=None):
    nc = bacc.Bacc("TRN2", target_bir_lowering=False, debug=False,
                   num_devices=NCORES)

    enc_d = nc.dram_tensor("enc", [TE, DE], F32, kind="ExternalInput").ap()
    dec_d = nc.dram_tensor("dec", [T, DD], F32, kind="ExternalInput").ap()
    ua_d = nc.dram_tensor("Ua", [DD, DE], F32, kind="ExternalInput").ap()
    wa_d = nc.dram_tensor("Wa", [DD, DE], F32, kind="ExternalInput").ap()
    va_d = nc.dram_tensor("Va", [DE], F32, kind="ExternalInput").ap()
    ctx_d = nc.dram_tensor("ctx_out", [T, DE], F32, kind="ExternalOutput").ap()
    attn_d = nc.dram_tensor("attn_out", [T, TE], F32, kind="ExternalOutput").ap()

    with tile.TileContext(nc) as tc, ExitStack() as ctx:
        if loop_iters is not None:
            ET = mybir.EngineType
            ctx.enter_context(tc.For_i(0, loop_iters, 1, hint_engines=(
                ET.PE, ET.Activation, ET.DVE, ET.Pool, ET.SP)))
        persist = ctx.enter_context(tc.tile_pool(name="persist", bufs=1))
        argp = ctx.enter_context(tc.tile_pool(name="argp", bufs=3))
        tanhp = ctx.enter_context(tc.tile_pool(name="tanhp", bufs=8))
        psum = ctx.enter_context(tc.tile_pool(name="psum", bufs=2, space="PSUM"))
        epsum = ctx.enter_context(tc.tile_pool(name="epsum", bufs=6, space="PSUM"))
        stagep = ctx.enter_context(tc.tile_pool(name="stagep", bufs=6))

        # ---- load inputs ----
        enc_sb = [persist.tile([128, DE], F32, tag=f"enc{j}", name=f"enc{j}") for j in range(2)]
        for j in range(2):
            nc.sync.dma_start(enc_sb[j][:], enc_d[j * 128:(j + 1) * 128, :])
        dec_sb = persist.tile([128, DD], F32, tag="dec", name="dec")
        nc.sync.dma_start(dec_sb[:], dec_d[:])
        ua_sb = [persist.tile([128, DE], F32, tag=f"ua{c}", name=f"ua{c}") for c in range(C)]
        wa_sb = [persist.tile([128, DE], F32, tag=f"wa{c}", name=f"wa{c}") for c in range(C)]
        for c in range(C):
            nc.sync.dma_start(ua_sb[c][:], ua_d[c * 128:(c + 1) * 128, :])
            nc.sync.dma_start(wa_sb[c][:], wa_d[c * 128:(c + 1) * 128, :])
        v_sb = persist.tile([128, C], F32, tag="v", name="v")
        nc.sync.dma_start(v_sb[:], va_d.rearrange("(c p) -> p c", p=128))

        ident = persist.tile([128, 128], F32, tag="ident", name="ident")
        make_identity(nc, ident[:])

        # ---- transposes: encT[c][cp, j] = enc[j, c*128+cp], decT likewise ----
        encT = [persist.tile([128, TE], F32, tag=f"encT{c}", name=f"encT{c}") for c in range(C)]
        decT = [persist.tile([128, T], F32, tag=f"decT{c}", name=f"decT{c}") for c in range(C)]
        for c in range(C):
            for j in range(2):
                tp = psum.tile([128, 128], F32, tag="ps", name="ps")
                nc.tensor.transpose(tp[:], enc_sb[j][:, c * 128:(c + 1) * 128],
                                    ident[:])
                nc.vector.tensor_copy(encT[c][:, j * 128:(j + 1) * 128], tp[:])
            tp = psum.tile([128, 128], F32, tag="ps", name="ps")
            nc.tensor.transpose(tp[:], dec_sb[:, c * 128:(c + 1) * 128], ident[:])
            nc.vector.tensor_copy(decT[c][:], tp[:])

        # ---- UhT[dc][dp, j] = Uh[j, dc*128+dp]; WsT[dc][dp, t] ----
        uhT = [persist.tile([128, TE], F16, tag=f"uhT{c}", name=f"uhT{c}") for c in range(C)]
        wsT = [persist.tile([128, T], F16, tag=f"wsT{c}", name=f"wsT{c}") for c in range(C)]
        for dc in range(C):
            pu = psum.tile([128, TE], F32, tag="ps", name="ps")
            for cc in range(C):
                nc.tensor.matmul(pu[:], ua_sb[cc][:, dc * 128:(dc + 1) * 128],
                                 encT[cc][:], start=(cc == 0), stop=(cc == C - 1))
            nc.vector.tensor_copy(uhT[dc][:], pu[:])
            pw = psum.tile([128, T], F32, tag="ps", name="ps")
            for cc in range(C):
                nc.tensor.matmul(pw[:], wa_sb[cc][:, dc * 128:(dc + 1) * 128],
                                 decT[cc][:], start=(cc == 0), stop=(cc == C - 1))
            nc.vector.tensor_copy(wsT[dc][:], pw[:])

        # ---- main loop: energies E[t, j] ----
        # fp16 args: WsT expanded (t-major, JB repeats) once per chunk on
        # GPSIMD so both DVE tensor_add operands have innermost step 1 ->
        # 2x_1P mode. ACT reads fp16 args, writes tanh as float32r; PE
        # V-dot: stationary = f32r V chunk [128d, 1] (M=1, trivial
        # LDWEIGHTS), moving = contiguous f32r tanh slice [128d, 512] at
        # 1 cyc/row, d-chunks accumulated in PSUM rows [1, 512]. Rows are
        # staged PSUM->SBUF by DVE copies software-pipelined 4 rows behind
        # the matmuls (strict-FIFO engines never head-of-line block), then
        # tiny SBUF->SBUF DMAs departition each row into E_sb[t, j].
        v_r = persist.tile([128, C], mybir.dt.float32r, tag="v_r", name="v_r")
        nc.vector.tensor_copy(v_r[:], v_sb[:])
        wse = [persist.tile([128, T * JB], F16, tag=f"wse{c}", name=f"wse{c}")
               for c in range(C)]
        for c in range(C):
            nc.gpsimd.tensor_copy(
                wse[c][:].rearrange("p (t r) -> p t r", t=T),
                wsT[c][:].unsqueeze(2).to_broadcast([128, T, JB]))
        e_sb = persist.tile([128, TE], F32, tag="e_sb", name="e_sb")
        TB = 512 // JB            # t's per PE matmul (N = TB*JB = 512)
        pending = []

        def drain_row():
            e_row, jb0, tb0 = pending.pop(0)
            stage = stagep.tile([1, TB * JB], F32, tag="stage", name="stage")
            nc.vector.tensor_copy(stage[:], e_row[0:1, :])
            nc.sync.dma_start(
                e_sb[tb0 * TB:(tb0 + 1) * TB, jb0 * JB:(jb0 + 1) * JB],
                stage[:].rearrange("o (t j) -> o t j", t=TB))

        for jb in range(NJB):
            tanhs = []
            for c in range(C):
                argt = argp.tile([128, T * JB], F16, tag="arg", name="arg")
                a3 = argt[:].rearrange("p (t j) -> p t j", t=T)
                w3 = wse[c][:].rearrange("p (t r) -> p t r", t=T)
                u3 = (uhT[c][:, jb * JB:(jb + 1) * JB]
                      .unsqueeze(1).to_broadcast([128, T, JB]))
                nc.vector.tensor_add(a3, w3, u3)
                tanht = tanhp.tile([128, T * JB], mybir.dt.float32r,
                                   tag="tanh", name="tanh")
                nc.scalar.activation(tanht[:], argt[:], AF.Tanh)
                tanhs.append(tanht)
            for tb in range(T // TB):
                while len(pending) >= 4:
                    drain_row()
                e_row = epsum.tile([128, TB * JB], F32, tag="e_row",
                                   name="e_row")
                for c in range(C):
                    # contiguous [128, 512] slice: t in [tb*TB, (tb+1)*TB),
                    # all JB j's of this tile, j-minor
                    rhs = tanhs[c][:, tb * TB * JB:(tb + 1) * TB * JB]
                    nc.tensor.matmul(e_row[0:1, :], v_r[:, c:c + 1], rhs,
                                     start=(c == 0), stop=(c == C - 1))
                pending.append((e_row, jb, tb))
        while pending:
            drain_row()

        nmx = persist.tile([128, 1], F32, tag="nmx", name="nmx")
        nc.vector.tensor_reduce(nmx[:], e_sb[:], axis=mybir.AxisListType.X,
                                op=mybir.AluOpType.max, negate=True)
        p_sb = persist.tile([128, TE], F32, tag="p_sb", name="p_sb")
        ssum = persist.tile([128, 1], F32, tag="ssum", name="ssum")
        nc.scalar.activation(p_sb[:], e_sb[:], AF.Exp, bias=nmx[:],
                             accum_out=ssum[:])
        rs = persist.tile([128, 1], F32, tag="rs", name="rs")
        nc.vector.reciprocal(rs[:], ssum[:])
        attn_sb = persist.tile([128, TE], F32, tag="attn_sb", name="attn_sb")
        nc.vector.tensor_scalar_mul(attn_sb[:], p_sb[:], rs[:])
        nc.sync.dma_start(attn_d[:], attn_sb[:])

        # ---- context = attn @ enc ----
        attnT = [persist.tile([128, 128], F32, tag=f"attnT{j}", name=f"attnT{j}") for j in range(2)]
        for j in range(2):
            tp = psum.tile([128, 128], F32, tag="ps", name="ps")
            nc.tensor.transpose(tp[:], attn_sb[:, j * 128:(j + 1) * 128], ident[:])
            nc.vector.tensor_copy(attnT[j][:], tp[:])
        cps = psum.tile([128, DE], F32, tag="ps", name="ps")
        for j in range(2):
            nc.tensor.matmul(cps[:], attnT[j][:], enc_sb[j][:],
                             start=(j == 0), stop=(j == 1))
        ctx_sb = persist.tile([128, DE], F32, tag="ctx_sb", name="ctx_sb")
        nc.vector.tensor_copy(ctx_sb[:], cps[:])
        nc.sync.dma_start(ctx_d[:], ctx_sb[:])

    nc.compile()
    return nc


def _get_program(loop_iters=None):
    key = ("nc", loop_iters)
    if key not in _CACHE:
        _CACHE[key] = _build_program(loop_iters)
    return _CACHE[key]


def _make_in_maps(encoder_outputs, decoder_outputs, U_a, W_a, V_a):
    enc = np.ascontiguousarray(np.asarray(encoder_outputs, dtype=np.float32))
    dec = np.ascontiguousarray(np.asarray(decoder_outputs, dtype=np.float32))
    ua = np.ascontiguousarray(np.asarray(U_a, dtype=np.float32))
    wa = np.ascontiguousarray(np.asarray(W_a, dtype=np.float32))
    va = np.ascontiguousarray(np.asarray(V_a, dtype=np.float32))
    in_maps = []
    for core in range(NCORES):
        b, th = divmod(core, 2)
        in_maps.append({
            "enc": enc[b],
            "dec": np.ascontiguousarray(dec[b, th * T:(th + 1) * T]),
            "Ua": ua,
            "Wa": wa,
            "Va": va,
        })
    return in_maps


def run(encoder_outputs, decoder_outputs, U_a, W_a, V_a, **run_kwargs):
    """Compile (cached), run on 8 cores, gather. Returns (results, context, attn)."""
    nc = _get_program()
    in_maps = _make_in_maps(encoder_outputs, decoder_outputs, U_a, W_a, V_a)
    res = run_bass_kernel_spmd(nc, in_maps, core_ids=list(range(NCORES)),
                               **run_kwargs)
    context = np.empty((B, TD, DE), np.float32)
    attn = np.empty((B, TD, TE), np.float32)
    for core in range(NCORES):
        b, th = divmod(core, 2)
        context[b, th * T:(th + 1) * T] = res.results[core]["ctx_out"]
        attn[b, th * T:(th + 1) * T] = res.results[core]["attn_out"]
    return res, context, attn


def kernel(encoder_outputs, decoder_outputs, U_a, W_a, V_a):
    _, context, attn = run(encoder_outputs, decoder_outputs, U_a, W_a, V_a)
    return context, attn
